# revision 72
# baseline (speedup 1.0000x reference)
"""CrossScaleSelectiveScan Trainium2 Bass kernel.

Sharding: data-parallel over batch B=8 -> one batch per NeuronCore.
Per core: bilinear resizes folded into 1x1-conv matmuls (separable
tap kernels as strided-view matmul accumulation, exact edges via
scaled-identity corrections), two 64-step GRU scans run as independent
latency-chains (vertical + horizontal), then gating + output projection
+ residual. All matmuls bf16 with fp32 PSUM accumulation; gate biases
ride the ACT bias / scalar_tensor_tensor scalar slots.

Schedule notes (cost-model driven):
- the critical path is the W scan: its step t consumes COLUMN t of x
  (all 64 rows), so x is produced twice-over: rows 0-31 as row-chunks
  (feeding early H steps while l streams in), rows 32-63 as 8-column
  PANELS so panel 0 completes right after the last l tile lands and
  the W chain starts ~20us in (vs ~49us when x was row-only).
- inputs are cast to bf16 on HOST, so every load is cast-free and runs
  on the SP HWDGE queue (serial DMA device, order = critical path:
  P1a weights, l0, m0, s, bias, H-scan weights, l1, m1, l2, m2, m3,
  l3, W-scan weights, P3 weights). Pool never does DMA descriptor gen.
- the head interleaves P1a/P1b/x row-chunks by l-tile arrival and
  weaves in the first LEAD H steps; the u-loop pairs H step u with
  W step u-LEAD and weaves the x panels + late gxn chunks + P3 stages.
- out tiles are f32 so the output DMAs ride SP as well.
- deep tile-pool rotation (sc=32/sc2=24) removes WAR stalls from the
  scan chains; all PSUM->SBUF copies live on DVE so the ACT queue only
  carries chain sigmoids/tanhs (plus x relus).
"""
import numpy as np
import ml_dtypes
from contextlib import ExitStack

import concourse.bacc as bacc
import concourse.bass as bass
import concourse.mybir as mybir
import concourse.tile as tile
from concourse.bass_utils import run_bass_kernel_spmd

BF = mybir.dt.bfloat16
F32 = mybir.dt.float32
AF = mybir.ActivationFunctionType
ALU = mybir.AluOpType
NP_BF16 = ml_dtypes.bfloat16

C = 128
H = W = 64
T = 64
PX = H * W          # 4096
HL = WL = 128       # l spatial
HS = WS = 32        # s spatial
NT = 27             # weight tiles in bundle

# weight-bundle tile indices (grouped by DMA slice)
WL38, WL18, WL37, WL17 = 0, 1, 2, 3
WS34, WS14, WM = 4, 5, 6
ID18, ID38, ID356, ID156, ID34, ID14 = 7, 8, 9, 10, 11, 12
NA = 13                 # wbA tile count (head weights)
WIH_H, WHH_H = 13, 16   # +0 r, +1 z, +2 n
WIH_W, WHH_W = 19, 22
GW, PO = 25, 26

# bias columns
B_SHIFT_IN, B_GATE, B_SHIFT_OUT = 0, 1, 2
B_R_H, B_Z_H, B_HHN_H, B_IHN_H = 3, 4, 5, 6
B_R_W, B_Z_W, B_HHN_W, B_IHN_W = 7, 8, 9, 10
NB = 11


def _prep_shared(inp):
    """Build the per-core weight bundle (identical on every core)."""
    f = np.float32
    scale_i = inp['proj_in_scale'].astype(f)
    w_in = inp['proj_in_w'].astype(f) * scale_i[:, None]
    Wl, Wm, Ws = w_in[:, :C], w_in[:, C:2 * C], w_in[:, 2 * C:]
    eye = np.eye(C, dtype=f)

    tiles = [None] * NT
    for idx, k in ((WL18, 1 / 8), (WL38, 3 / 8), (WL37, 3 / 7), (WL17, 1 / 7)):
        tiles[idx] = (k * Wl).T
    for idx, k in ((ID18, 1 / 8), (ID38, 3 / 8), (ID356, 3 / 56),
                   (ID156, 1 / 56), (ID34, 3 / 4), (ID14, 1 / 4)):
        tiles[idx] = k * eye
    for idx, k in ((WS34, 3 / 4), (WS14, 1 / 4)):
        tiles[idx] = (k * Ws).T
    tiles[WM] = Wm.T
    for base, wname in ((WIH_H, 'wih_h'), (WHH_H, 'whh_h'),
                        (WIH_W, 'wih_w'), (WHH_W, 'whh_w')):
        wmat = inp[wname].astype(f)                          # [3C, C]
        for g in range(3):
            tiles[base + g] = wmat[g * C:(g + 1) * C, :].T
    tiles[GW] = inp['gate_w'].astype(f).T
    tiles[PO] = (inp['proj_out_w'].astype(f)
                 * inp['proj_out_scale'].astype(f)[:, None]).T

    wb = np.concatenate(tiles, axis=1).astype(NP_BF16)       # [128, NT*128]

    bih_h, bhh_h = inp['bih_h'].astype(f), inp['bhh_h'].astype(f)
    bih_w, bhh_w = inp['bih_w'].astype(f), inp['bhh_w'].astype(f)
    bias = np.zeros((C, NB), f)
    bias[:, B_SHIFT_IN] = inp['proj_in_shift'].astype(f)
    bias[:, B_GATE] = inp['gate_b'].astype(f)
    bias[:, B_SHIFT_OUT] = inp['proj_out_shift'].astype(f)
    bias[:, B_R_H] = bih_h[:C] + bhh_h[:C]
    bias[:, B_Z_H] = bih_h[C:2 * C] + bhh_h[C:2 * C]
    bias[:, B_HHN_H] = bhh_h[2 * C:]
    bias[:, B_IHN_H] = bih_h[2 * C:]
    bias[:, B_R_W] = bih_w[:C] + bhh_w[:C]
    bias[:, B_Z_W] = bih_w[C:2 * C] + bhh_w[C:2 * C]
    bias[:, B_HHN_W] = bhh_w[2 * C:]
    bias[:, B_IHN_W] = bih_w[2 * C:]
    return wb, bias


def build_nc(loop_n=1):
    nc = bacc.Bacc("TRN2", target_bir_lowering=False)
    l_d = nc.dram_tensor("l", [C, HL * WL], BF, kind="ExternalInput")
    m_d = nc.dram_tensor("m", [C, PX], BF, kind="ExternalInput")
    s_d = nc.dram_tensor("s", [C, HS * WS], BF, kind="ExternalInput")
    wb_d = nc.dram_tensor("wb", [C, NT * C], BF, kind="ExternalInput")
    bias_d = nc.dram_tensor("bias", [C, NB], F32, kind="ExternalInput")
    out_d = nc.dram_tensor("out", [C, PX], F32, kind="ExternalOutput")

    with tile.TileContext(nc) as tc, ExitStack() as ctx:
        big = ctx.enter_context(tc.tile_pool(name="big", bufs=1))
        l_ts = [big.tile([C, 32 * WL], BF, name=f"lt{i}", tag=f"l{i}")
                for i in range(4)]
        m_sb = big.tile([C, PX], BF, tag="m")
        s_sb = big.tile([C, HS * WS], BF, tag="s")
        wb = big.tile([C, NT * C], BF, tag="wb")
        bias = big.tile([C, NB], F32, tag="bias")
        o1 = big.tile([C, HL * W], BF, tag="o1")      # (h=128, w'=64)
        o1s = big.tile([C, HS * W], BF, tag="o1s")    # (hs=32, w'=64)
        x_sb = big.tile([C, PX], BF, tag="x")
        gxn_t = {('h', c): big.tile([C, 512], BF, name=f"gxnh{c}", tag=f"gxnh{c}")
                 for c in range(8)}
        gxn_t.update({('w', c): big.tile([C, 512], BF, name=f"gxnw{c}", tag=f"gxnw{c}")
                      for c in range(8)})
        oh = big.tile([C, PX], BF, tag="oh")          # (h, w)
        ow_ts = [big.tile([C, 512], BF, name=f"owt{i}", tag=f"owt{i}")
                 for i in range(8)]                   # (w, h) in 8 w-chunks
        h0 = big.tile([C, 64], BF, tag="h0")
        scd = big.tile([C, PX], BF, tag="scd")
        out_ts = [big.tile([C, 1024], F32, name=f"outsb{i}", tag=f"outsb{i}")
                  for i in range(4)]

        nc.vector.memset(h0[:], 0.0)
        actwarm = big.tile([C, 3], BF, tag="actwarm")
        nc.scalar.activation(actwarm[:, 0:1], h0[:, 0:1], AF.Sigmoid)
        nc.scalar.activation(actwarm[:, 1:2], h0[:, 1:2], AF.Tanh)
        nc.scalar.activation(actwarm[:, 2:3], h0[:, 2:3], AF.Relu)

        def wt(i):
            return wb[:, i * C:(i + 1) * C]

        def bcol(i):
            return bias[:, i:i + 1]

        sv = s_sb[:].rearrange("p (h w) -> p h w", h=HS)
        o1v = o1[:].rearrange("p (h w) -> p h w", h=HL)
        o1sv = o1s[:].rearrange("p (h w) -> p h w", h=HS)
        xv = x_sb[:].rearrange("p (h w) -> p h w", h=H)
        mv = m_sb[:].rearrange("p (h w) -> p h w", h=H)
        mm = nc.tensor.matmul
        dma = nc.sync.dma_start

        for _it in range(loop_n):
            # ---- loads: cast-free (host pre-casts to bf16) on the SP HWDGE
            # queue. The DMA device is serial in the model, so ORDER = the
            # critical path.
            dma(wb[:, 0:2 * C], wb_d[:, 0:2 * C])     # P1a chunk weights
            dma(l_ts[0][:], l_d[:, 0:4096])
            dma(wb[:, 2 * C:NA * C], wb_d[:, 2 * C:NA * C])
            dma(m_sb[:, 0:1024], m_d[:, 0:1024])
            dma(s_sb[:], s_d[:])
            dma(bias[:], bias_d[:])
            dma(wb[:, NA * C:(NA + 6) * C], wb_d[:, NA * C:(NA + 6) * C])
            dma(l_ts[1][:], l_d[:, 4096:8192])
            dma(m_sb[:, 1024:2048], m_d[:, 1024:2048])
            dma(l_ts[2][:], l_d[:, 8192:12288])
            dma(l_ts[3][:], l_d[:, 12288:16384])
            dma(m_sb[:, 2048:3072], m_d[:, 2048:3072])
            dma(m_sb[:, 3072:4096], m_d[:, 3072:4096])
            dma(wb[:, (NA + 6) * C:(NA + 12) * C],
                wb_d[:, (NA + 6) * C:(NA + 12) * C])
            dma(wb[:, (NA + 12) * C:], wb_d[:, (NA + 12) * C:])
            lvs = [t[:].rearrange("p (h w) -> p h w", h=32) for t in l_ts]

            LEAD = 6

            with tc.tile_pool(name="psn", bufs=1, space="PSUM") as psn, \
                 tc.tile_pool(name="sc", bufs=32) as sc, \
                 tc.tile_pool(name="sc2", bufs=24) as sc2, \
                 tc.tile_pool(name="p3", bufs=4) as p3:
                psH_cm = tc.tile_pool(name="psH", bufs=1, space="PSUM")
                psH = psH_cm.__enter__()
                P = {}   # late-bound pools (psx/ps1/pse in head, psW/p3d after)

                def p1a_chunk(k):
                    lt = lvs[k // 4]
                    r0 = 8 * (k % 4)
                    p = P['ps1'].tile([C, 512], F32, tag="o1p", name=f"o1p{k}")
                    pvv = p[:].rearrange("p (h w) -> p h w", h=8)
                    rows = lt[:, r0:r0 + 8, :]
                    mm(pvv[:, :, :], wt(WL38), rows[:, :, 0:128:2], start=True, stop=False, skip_group_check=True)
                    mm(pvv[:, :, :], wt(WL38), rows[:, :, 1:128:2], start=False, stop=False, skip_group_check=True)
                    mm(pvv[:, :, 1:64], wt(WL18), rows[:, :, 1:126:2], start=False, stop=False, skip_group_check=True)
                    mm(pvv[:, :, 0:63], wt(WL18), rows[:, :, 2:127:2], start=False, stop=True, skip_group_check=True)
                    # copy only cols 1..62: cols 0/63 belong to the exact
                    # edge pass, whose copies are order-independent this way.
                    # Late chunks alternate DVE/ACT so the copy stream (which
                    # gates the x panels and the W scan) is not DVE-bound.
                    if k >= 8 and k % 2 == 1:
                        nc.scalar.activation(o1v[:, 8 * k:8 * k + 8, 1:63],
                                             pvv[:, :, 1:63], AF.Copy)
                    else:
                        nc.vector.tensor_copy(o1v[:, 8 * k:8 * k + 8, 1:63],
                                              pvv[:, :, 1:63])

                def p1a_edge(lti):
                    # exact edge columns w'=0 / w'=63 for l-tile lti's rows
                    ep_t = P['pse'].tile([C, 512], F32, tag="ep", name=f"ep{lti}")
                    epv = ep_t[:, 0:256].rearrange("p (e h) -> p e h", e=2)
                    for i, (wcol, widx) in enumerate(((0, WL37), (1, WL37), (2, WL17))):
                        mm(epv[:, 0, 32 * lti:32 * lti + 32], wt(widx),
                           lvs[lti][:, :, wcol], start=(i == 0),
                           stop=(i == 2), skip_group_check=True)
                    for i, (wcol, widx) in enumerate(((125, WL17), (126, WL37), (127, WL37))):
                        mm(epv[:, 1, 32 * lti:32 * lti + 32], wt(widx),
                           lvs[lti][:, :, wcol], start=(i == 0),
                           stop=(i == 2), skip_group_check=True)
                    nc.vector.tensor_copy(o1v[:, 32 * lti:32 * lti + 32, 0],
                                          epv[:, 0, 32 * lti:32 * lti + 32])
                    nc.vector.tensor_copy(o1v[:, 32 * lti:32 * lti + 32, 63],
                                          epv[:, 1, 32 * lti:32 * lti + 32])

                def p1b_chunk(k):
                    # s W-upsample fused with conv -> o1s rows 8k..8k+7
                    r0 = 8 * k
                    p = P['pse'].tile([C, 512], F32, tag="ep", name=f"o1sp{k}")
                    pvv = p[:].rearrange("p (h w) -> p h w", h=8)
                    srows = sv[:, r0:r0 + 8, :]
                    mm(pvv[:, :, 0:64:2], wt(WS34), srows[:, :, 0:32], start=True, stop=False, skip_group_check=True)
                    mm(pvv[:, :, 1:64:2], wt(WS34), srows[:, :, 0:32], start=False, stop=False, skip_group_check=True)
                    mm(pvv[:, :, 2:64:2], wt(WS14), srows[:, :, 0:31], start=False, stop=False, skip_group_check=True)
                    mm(pvv[:, :, 1:63:2], wt(WS14), srows[:, :, 1:32], start=False, stop=False, skip_group_check=True)
                    mm(pvv[:, :, 0], wt(WS14), srows[:, :, 0], start=False, stop=False, skip_group_check=True)
                    mm(pvv[:, :, 63], wt(WS14), srows[:, :, 31], start=False, stop=True, skip_group_check=True)
                    # PSUM evacuation on ACT: keeps the head DVE free for
                    # the o1p copies that gate the x panels
                    nc.scalar.activation(o1s[:, k * 512:(k + 1) * 512], p[:],
                                         AF.Copy)

                def x_chunk(c):
                    # x rows 8c..8c+7, all 64 cols (c = 0..3 only)
                    hp0 = 8 * c
                    p = P['psx'].tile([C, 512], F32, tag="xp", name=f"xp{c}")
                    pvv = p[:].rearrange("p (h w) -> p h w", h=8)
                    mm(p[:], wt(WM), m_sb[:, c * 512:(c + 1) * 512], start=True, stop=False, skip_group_check=True)
                    mm(pvv[:, :, :], wt(ID38), o1v[:, 2 * hp0:2 * hp0 + 16:2, :], start=False, stop=False, skip_group_check=True)
                    mm(pvv[:, :, :], wt(ID38), o1v[:, 2 * hp0 + 1:2 * hp0 + 16:2, :], start=False, stop=False, skip_group_check=True)
                    if c == 0:
                        mm(pvv[:, 1:8, :], wt(ID18), o1v[:, 1:15:2, :], start=False, stop=False, skip_group_check=True)
                    else:
                        mm(pvv[:, :, :], wt(ID18), o1v[:, 2 * hp0 - 1:2 * hp0 + 15:2, :], start=False, stop=False, skip_group_check=True)
                    mm(pvv[:, :, :], wt(ID18), o1v[:, 2 * hp0 + 2:2 * hp0 + 18:2, :], start=False, stop=False, skip_group_check=True)
                    if c == 0:
                        mm(pvv[:, 0, :], wt(ID356), o1v[:, 0, :], start=False, stop=False, skip_group_check=True)
                        mm(pvv[:, 0, :], wt(ID356), o1v[:, 1, :], start=False, stop=False, skip_group_check=True)
                        mm(pvv[:, 0, :], wt(ID156), o1v[:, 2, :], start=False, stop=False, skip_group_check=True)
                    p0 = 4 * c
                    mm(pvv[:, 0:8:2, :], wt(ID34), o1sv[:, p0:p0 + 4, :], start=False, stop=False, skip_group_check=True)
                    mm(pvv[:, 1:8:2, :], wt(ID34), o1sv[:, p0:p0 + 4, :], start=False, stop=False, skip_group_check=True)
                    if c == 0:
                        mm(pvv[:, 2:8:2, :], wt(ID14), o1sv[:, 0:3, :], start=False, stop=False, skip_group_check=True)
                        mm(pvv[:, 0, :], wt(ID14), o1sv[:, 0, :], start=False, stop=False, skip_group_check=True)
                    else:
                        mm(pvv[:, 0:8:2, :], wt(ID14), o1sv[:, p0 - 1:p0 + 3, :], start=False, stop=False, skip_group_check=True)
                    mm(pvv[:, 1:8:2, :], wt(ID14), o1sv[:, p0 + 1:p0 + 5, :], start=False, stop=(c != 0), skip_group_check=True)
                    nc.scalar.activation(x_sb[:, c * 512:(c + 1) * 512], p[:],
                                         AF.Relu, bias=bcol(B_SHIFT_IN))
                    # gxn for the H scan streams right behind each x chunk;
                    # the bias-add rides ACT (Identity+bias) in the head
                    pg = psn.tile([C, 512], F32, tag="gxnp", name=f"gxnp{c}")
                    mm(pg[:], wt(WIH_H + 2), x_sb[:, c * 512:(c + 1) * 512],
                       start=True, stop=True, skip_group_check=True)
                    nc.scalar.activation(gxn_t[('h', c)][:], pg[:],
                                         AF.Identity, bias=bcol(B_IHN_H))

                def gxn_h_chunk(c):
                    # late H gxn chunks (rows from the x panels), c = 4..7;
                    # DVE halves (ACT carries both chains in the u-loop)
                    pg = psn.tile([C, 512], F32, tag="gxnp", name=f"gxnp{c}")
                    mm(pg[:], wt(WIH_H + 2), x_sb[:, c * 512:(c + 1) * 512],
                       start=True, stop=True, skip_group_check=True)
                    for hh in range(2):
                        nc.vector.tensor_scalar(
                            gxn_t[('h', c)][:, hh * 256:(hh + 1) * 256],
                            pg[:, hh * 256:(hh + 1) * 256],
                            bcol(B_IHN_H), None, ALU.add)

                def x_panel(p):
                    # x rows 32..63 for cols 8p..8p+8 (feeds W steps early);
                    # PSUM rides the psn ring (alternates with gxn chunks)
                    j0 = 8 * p
                    pp = psn.tile([C, 512], F32, tag="gxnp", name=f"xpp{p}")
                    pv = pp[:, 0:256].rearrange("p (h w) -> p h w", h=32)
                    mm(pv[:, :, :], wt(WM), mv[:, 32:64, j0:j0 + 8], start=True, stop=False, skip_group_check=True)
                    mm(pv[:, :, :], wt(ID38), o1v[:, 64:128:2, j0:j0 + 8], start=False, stop=False, skip_group_check=True)
                    mm(pv[:, :, :], wt(ID38), o1v[:, 65:128:2, j0:j0 + 8], start=False, stop=False, skip_group_check=True)
                    mm(pv[:, :, :], wt(ID18), o1v[:, 63:127:2, j0:j0 + 8], start=False, stop=False, skip_group_check=True)
                    mm(pv[:, 0:31, :], wt(ID18), o1v[:, 66:128:2, j0:j0 + 8], start=False, stop=False, skip_group_check=True)
                    mm(pv[:, 31, :], wt(ID156), o1v[:, 125, j0:j0 + 8], start=False, stop=False, skip_group_check=True)
                    mm(pv[:, 31, :], wt(ID356), o1v[:, 126, j0:j0 + 8], start=False, stop=False, skip_group_check=True)
                    mm(pv[:, 31, :], wt(ID356), o1v[:, 127, j0:j0 + 8], start=False, stop=False, skip_group_check=True)
                    mm(pv[:, 0:32:2, :], wt(ID34), o1sv[:, 16:32, j0:j0 + 8], start=False, stop=False, skip_group_check=True)
                    mm(pv[:, 1:32:2, :], wt(ID34), o1sv[:, 16:32, j0:j0 + 8], start=False, stop=False, skip_group_check=True)
                    mm(pv[:, 0:32:2, :], wt(ID14), o1sv[:, 15:31, j0:j0 + 8], start=False, stop=False, skip_group_check=True)
                    mm(pv[:, 1:31:2, :], wt(ID14), o1sv[:, 17:32, j0:j0 + 8], start=False, stop=False, skip_group_check=True)
                    mm(pv[:, 31, :], wt(ID14), o1sv[:, 31, j0:j0 + 8], start=False, stop=True, skip_group_check=True)
                    nc.scalar.activation(xv[:, 32:64, j0:j0 + 8], pv[:, :, :],
                                         AF.Relu, bias=bcol(B_SHIFT_IN))

                def gxn_w_chunk(c):
                    pg = psn.tile([C, 512], F32, tag="gxnp", name=f"gxnw{c}")
                    rhs = xv[:, :, 8 * c:8 * c + 8].transpose([0, 2, 1])
                    mm(pg[:].rearrange("p (w h) -> p w h", w=8), wt(WIH_W + 2), rhs,
                       start=True, stop=True, skip_group_check=True)
                    # bias-add in [C,256] halves (GPSIMD has no PSUM port, so
                    # these stay on DVE; halves bound the chain-delay spill)
                    for hh in range(2):
                        nc.vector.tensor_scalar(
                            gxn_t[('w', c)][:, hh * 256:(hh + 1) * 256],
                            pg[:, hh * 256:(hh + 1) * 256],
                            bcol(B_IHN_W), None, ALU.add)

                scans = {
                    'h': ('h', WIH_H, WHH_H, B_R_H, B_Z_H, B_HHN_H),
                    'w': ('w', WIH_W, WHH_W, B_R_W, B_Z_W, B_HHN_W),
                }
                spool = {'h': psH}

                def hbuf(sname, t):
                    if sname == 'h':
                        return oh[:, t * 64:(t + 1) * 64]
                    return ow_ts[t // 8][:, (t % 8) * 64:(t % 8) * 64 + 64]
                Pcur = {}
                Pcur2 = {}
                prev = {'h': None, 'w': None}

                def scan_mms(sname, t):
                    buf, wih, whh = scans[sname][:3]
                    pool = spool[sname]
                    Pr_t = pool.tile([C, 64], F32, tag=f"Pr{sname}", name=f"Pr{sname}{t}")
                    Pz_t = pool.tile([C, 64], F32, tag=f"Pz{sname}", name=f"Pz{sname}{t}")
                    Pn_t = pool.tile([C, 64], F32, tag=f"Pn{sname}", name=f"Pn{sname}{t}")
                    Pr, Pz, Pn = Pr_t[:], Pz_t[:], Pn_t[:]
                    xt = (x_sb[:, t * 64:(t + 1) * 64]
                          if sname == 'h' else xv[:, :, t])
                    mm(Pr, wt(wih + 0), xt, start=True, stop=False, skip_group_check=True)
                    mm(Pz, wt(wih + 1), xt, start=True, stop=False, skip_group_check=True)
                    if prev[sname] is None:
                        mm(Pr, wt(whh + 0), h0[:], start=False, stop=True, skip_group_check=True)
                        mm(Pn, wt(whh + 2), h0[:], start=True, stop=True, skip_group_check=True)
                        mm(Pz, wt(whh + 1), h0[:], start=False, stop=True, skip_group_check=True)
                    else:
                        # h' = t1 + t2, and whh@h' = whh@t2 + whh@t1: the t2
                        # half issues early (t2 is ready before tanh), only
                        # the t1 half waits on the tanh chain.
                        t1p, t2p = prev[sname]
                        mm(Pr, wt(whh + 0), t2p[:], start=False, stop=False, skip_group_check=True)
                        mm(Pn, wt(whh + 2), t2p[:], start=True, stop=False, skip_group_check=True)
                        mm(Pz, wt(whh + 1), t2p[:], start=False, stop=False, skip_group_check=True)
                        mm(Pr, wt(whh + 0), t1p[:], start=False, stop=True, skip_group_check=True)
                        mm(Pn, wt(whh + 2), t1p[:], start=False, stop=True, skip_group_check=True)
                        mm(Pz, wt(whh + 1), t1p[:], start=False, stop=True, skip_group_check=True)
                    Pcur[sname] = (Pr, Pz, Pn)

                def scan_gates(sname, t):
                    buf, wih, whh, br, bz, bhhn = scans[sname]
                    Pr, Pz, Pn = Pcur[sname]
                    hp = h0[:] if t == 0 else hbuf(sname, t - 1)
                    r = sc.tile([C, 64], BF, tag=f"r{sname}", name=f"r{sname}{t}")
                    nc.scalar.activation(r[:], Pr, AF.Sigmoid, bias=bcol(br))
                    q = sc.tile([C, 64], BF, tag=f"q{sname}", name=f"q{sname}{t}")
                    nc.vector.scalar_tensor_tensor(q[:], Pn,
                                                   bcol(bhhn), r[:],
                                                   ALU.add, ALU.mult)
                    nin = sc.tile([C, 64], BF, tag=f"nin{sname}", name=f"nin{sname}{t}")
                    nc.vector.tensor_add(nin[:], q[:],
                                         gxn_t[(sname, t // 8)][:, (t % 8) * 64:(t % 8) * 64 + 64])
                    # sigma_z issues BEFORE tanh: it executes in the ACT
                    # gap while the STT/nin legs run, so zc (which gates the
                    # on-chain t1 multiply) is ready by the time tanh lands.
                    z = sc2.tile([C, 64], BF, tag=f"z{sname}", name=f"z{sname}{t}")
                    nc.scalar.activation(z[:], Pz, AF.Sigmoid, bias=bcol(bz))
                    zc = sc2.tile([C, 64], BF, tag=f"zc{sname}", name=f"zc{sname}{t}")
                    nc.gpsimd.tensor_scalar(zc[:], z[:], -1.0, 1.0,
                                            ALU.mult, ALU.add)
                    t2 = sc2.tile([C, 64], BF, tag=f"t2{sname}", name=f"t2{sname}{t}")
                    nc.gpsimd.tensor_mul(t2[:], z[:], hp)
                    n = sc.tile([C, 64], BF, tag=f"n{sname}", name=f"n{sname}{t}")
                    nc.scalar.activation(n[:], nin[:], AF.Tanh)
                    t1 = sc.tile([C, 64], BF, tag=f"t1{sname}", name=f"t1{sname}{t}")
                    nc.vector.tensor_mul(t1[:], n[:], zc[:])
                    nc.vector.tensor_add(hbuf(sname, t), t1[:], t2[:])
                    prev[sname] = (t1, t2)

                # ---- P3 stage helpers (issued in the W-tail + drain)
                ohv = oh[:].rearrange("p (h w) -> p h w", h=H)
                scdv = scd[:].rearrange("p (w h) -> p w h", w=W)

                def p3_stage0(c):
                    # scd = ow + oh^T in [C,256] halves, split DVE/Pool
                    for hh in range(2):
                        h2 = slice(c * 512 + hh * 256, c * 512 + (hh + 1) * 256)
                        oh_view = ohv[:, :, 8 * c + 4 * hh:8 * c + 4 * hh + 4] \
                            .transpose([0, 2, 1])
                        eng = nc.vector if hh == 0 else nc.gpsimd
                        eng.tensor_add(scd[:, h2],
                                       ow_ts[c][:, hh * 256:(hh + 1) * 256],
                                       oh_view)

                def p3_stage1(c):
                    # gp = GW@ow + GW@oh^T accumulated in PSUM: the gate path
                    # does not wait for the Pool scd add (scd only feeds the
                    # gated multiply in stage2)
                    gp = P['p3gp'].tile([C, 512], F32, tag="p3gp", name=f"gp{c}")
                    g = p3.tile([C, 512], BF, tag="g", name=f"g{c}")
                    for hh in range(2):
                        h2 = slice(hh * 256, (hh + 1) * 256)
                        oh_view = ohv[:, :, 8 * c + 4 * hh:8 * c + 4 * hh + 4] \
                            .transpose([0, 2, 1])
                        mm(gp[:, h2], wt(GW), ow_ts[c][:, hh * 256:(hh + 1) * 256],
                           start=True, stop=False, skip_group_check=True)
                        mm(gp[:, h2].rearrange("p (w h) -> p w h", w=4), wt(GW),
                           oh_view, start=False, stop=True, skip_group_check=True)
                        nc.scalar.activation(g[:, h2], gp[:, h2], AF.Sigmoid,
                                             bias=bcol(B_GATE))
                    return g

                def p3_stage2(c, g):
                    sl = slice(c * 512, (c + 1) * 512)
                    gated = p3.tile([C, 512], BF, tag="gated", name=f"gated{c}")
                    op = psn.tile([C, 512], F32, tag="gxnp", name=f"op{c}")
                    for hh in range(2):
                        h2 = slice(hh * 256, (hh + 1) * 256)
                        nc.vector.tensor_mul(gated[:, h2], scd[:, sl][:, h2], g[:, h2])
                        mm(op[:, h2], wt(PO), gated[:, h2],
                           start=True, stop=True, skip_group_check=True)
                    return op

                def p3_stage3(c, op):
                    y = p3.tile([C, 512], BF, tag="y", name=f"y{c}")
                    nc.vector.tensor_scalar(y[:, 0:256], op[:, 0:256],
                                            bcol(B_SHIFT_OUT), 0.0,
                                            ALU.add, ALU.max)
                    nc.scalar.activation(y[:, 256:512], op[:, 256:512],
                                         AF.Relu, bias=bcol(B_SHIFT_OUT))
                    return y

                def p3_cols(w0, w1):
                    # all P3 stages for w-cols [w0, w1) in one serial strip
                    # (used for the last chunk so only col 63 drains at the end)
                    nw = w1 - w0
                    sl = slice(w0 * 64, w1 * 64)
                    ow_v = ow_ts[w0 // 8][:, (w0 % 8) * 64:(w0 % 8) * 64 + nw * 64]
                    oh_view = ohv[:, :, w0:w1].transpose([0, 2, 1])
                    eng = nc.vector if (w0 % 2 == 0) else nc.gpsimd
                    eng.tensor_add(scd[:, sl], ow_v, oh_view)
                    gp = P['p3gp'].tile([C, 512], F32, tag="p3gp", name=f"gpc{w0}")
                    g = p3.tile([C, 512], BF, tag="g", name=f"gc{w0}")
                    mm(gp[:, 0:nw * 64], wt(GW), ow_v,
                       start=True, stop=False, skip_group_check=True)
                    mm(gp[:, 0:nw * 64].rearrange("p (w h) -> p w h", w=nw), wt(GW),
                       oh_view, start=False, stop=True, skip_group_check=True)
                    nc.scalar.activation(g[:, 0:nw * 64], gp[:, 0:nw * 64],
                                         AF.Sigmoid, bias=bcol(B_GATE))
                    gated = p3.tile([C, 512], BF, tag="gated", name=f"gatedc{w0}")
                    nc.vector.tensor_mul(gated[:, 0:nw * 64], scd[:, sl],
                                         g[:, 0:nw * 64])
                    op = psn.tile([C, 512], F32, tag="gxnp", name=f"opc{w0}")
                    mm(op[:, 0:nw * 64], wt(PO), gated[:, 0:nw * 64],
                       start=True, stop=True, skip_group_check=True)
                    y = p3.tile([C, 512], BF, tag="y", name=f"yc{w0}")
                    nc.vector.tensor_scalar(y[:, 0:nw * 64], op[:, 0:nw * 64],
                                            bcol(B_SHIFT_OUT), 0.0,
                                            ALU.add, ALU.max)
                    x_view = xv[:, :, w0:w1].transpose([0, 2, 1])
                    ot = out_ts[w0 // 16][:, (w0 % 16) * 64:(w0 % 16) * 64 + nw * 64]
                    nc.vector.tensor_add(ot, y[:, 0:nw * 64], x_view)
                    dma(out_d[:, sl], ot)

                def p3_stage4(c, y):
                    for hh in range(2):
                        h2 = slice(hh * 256, (hh + 1) * 256)
                        x_view = xv[:, :, 8 * c + 4 * hh:8 * c + 4 * hh + 4] \
                            .transpose([0, 2, 1])
                        nc.vector.tensor_add(
                            out_ts[c // 2][:, (c % 2) * 512 + hh * 256:
                                           (c % 2) * 512 + (hh + 1) * 256],
                            y[:, h2], x_view)
                    dma(out_d[:, c * 512:(c + 1) * 512],
                        out_ts[c // 2][:, (c % 2) * 512:(c % 2) * 512 + 512])

                # ===== HEAD: P1a/P1b/x row-chunks interleaved by l-tile
                # arrival, first LEAD H steps woven in.
                with tc.tile_pool(name="ps1", bufs=2, space="PSUM") as ps1, \
                     tc.tile_pool(name="pse", bufs=1, space="PSUM") as pse, \
                     tc.tile_pool(name="psx", bufs=1, space="PSUM") as psx:
                    P['ps1'], P['pse'], P['psx'] = ps1, pse, psx
                    # PE p-state warmup: the cost model needs ~3us of
                    # continuous PE busy to reach full clock; spin on h0
                    # until the l0-dependent work arrives.
                    warm = ps1.tile([C, 512], F32, tag="o1p", name="warm")
                    for _w in range(140):
                        mm(warm[:][0:64, 0:64], h0[:, 0:64], h0[:],
                           start=True, stop=True, skip_group_check=True)

                    # l0-gated work + first H steps
                    for k in range(4):
                        p1a_chunk(k)
                    p1a_edge(0)
                    p1b_chunk(0)
                    p1b_chunk(1)
                    x_chunk(0)
                    for t in (0, 1):
                        with tc.high_priority(offset=100000):
                            scan_mms('h', t)
                            scan_gates('h', t)
                    # l1-gated
                    for k in range(4, 8):
                        p1a_chunk(k)
                    p1a_edge(1)
                    x_chunk(1)
                    for t in (2, 3):
                        with tc.high_priority(offset=100000):
                            scan_mms('h', t)
                            scan_gates('h', t)
                    p1b_chunk(2)
                    p1b_chunk(3)
                    x_chunk(2)
                    # l2-gated
                    for k in range(8, 12):
                        p1a_chunk(k)
                    p1a_edge(2)
                    x_chunk(3)
                    for t in (4, 5):
                        with tc.high_priority(offset=100000):
                            scan_mms('h', t)
                            scan_gates('h', t)
                    # l3-gated
                    for k in range(12, 16):
                        p1a_chunk(k)
                    p1a_edge(3)
                    x_panel(0)
                    gxn_w_chunk(0)

                # ===== u-loop: paired H+W steps; x panels, late gxn chunks,
                # early scd halves and P3 stages woven in.
                with tc.tile_pool(name="psW", bufs=1, space="PSUM") as psW, \
                     tc.tile_pool(name="p3gp", bufs=1, space="PSUM") as p3gp:
                    spool['w'] = psW
                    P['p3gp'] = p3gp

                    # weave schedule: panel p (p>=1) at u, late gxn_h at u
                    panel_at = {5 + 2 * p: p for p in range(1, 8)}      # u=7..19
                    gxnh_at = {21 + 2 * i: 4 + i for i in range(4)}     # u=21..27

                    # P3: chunk c needs the FULL H-scan (u >= T) plus ow
                    # chunk c (u >= 8c+7+LEAD). Whatever does not fit in the
                    # u-loop window runs in the stage-major drain.
                    p3_sched = {}          # u -> list of (stage_idx, chunk)
                    p3_done = set()
                    for c3 in range(7):
                        for s in range(5):
                            uu = max(T, 8 * c3 + 7 + LEAD) + 2 * c3 + s
                            if uu < T + LEAD:
                                p3_sched.setdefault(uu, []).append((s, c3))
                                p3_done.add((s, c3))

                    p3g = {}
                    for u in range(LEAD, T + LEAD):
                        tH, tW = u, u - LEAD
                        with tc.high_priority(offset=100000):
                            if tH < T:
                                scan_mms('h', tH)
                            scan_mms('w', tW)
                            if tH < T:
                                scan_gates('h', tH)
                            scan_gates('w', tW)
                        if u in panel_at:
                            x_panel(panel_at[u])
                            gxn_w_chunk(panel_at[u])
                        if u in gxnh_at:
                            gxn_h_chunk(gxnh_at[u])
                        for (s, c3) in p3_sched.get(u, []):
                            if s == 0:
                                p3_stage0(c3)
                            elif s == 1:
                                p3g[c3] = p3_stage1(c3)
                            elif s == 2:
                                p3g[c3] = p3_stage2(c3, p3g[c3])
                            elif s == 3:
                                p3g[c3] = p3_stage3(c3, p3g[c3])
                            else:
                                p3_stage4(c3, p3g.pop(c3))
                    # stage-major drain of everything the u-loop didn't fit
                    for s in range(5):
                        for c3 in range(7):
                            if (s, c3) in p3_done:
                                continue
                            if s == 0:
                                p3_stage0(c3)
                            elif s == 1:
                                p3g[c3] = p3_stage1(c3)
                            elif s == 2:
                                p3g[c3] = p3_stage2(c3, p3g[c3])
                            elif s == 3:
                                p3g[c3] = p3_stage3(c3, p3g[c3])
                            else:
                                p3_stage4(c3, p3g.pop(c3))
                    p3_cols(56, 60)
                    p3_cols(60, 63)
                    p3_cols(63, 64)
                psH_cm.__exit__(None, None, None)

    nc.finalize()
    return nc


_NC_CACHE = {}


def kernel(**inputs):
    inputs = {k: np.asarray(v) for k, v in inputs.items()}
    B = inputs['l'].shape[0]
    wb, bias = _prep_shared(inputs)
    if 'nc' not in _NC_CACHE:
        _NC_CACHE['nc'] = build_nc()
    nc = _NC_CACHE['nc']
    in_maps = []
    for b in range(B):
        in_maps.append({
            'l': inputs['l'][b].reshape(C, -1).astype(NP_BF16),
            'm': inputs['m'][b].reshape(C, -1).astype(NP_BF16),
            's': inputs['s'][b].reshape(C, -1).astype(NP_BF16),
            'wb': wb, 'bias': bias,
        })
    res = run_bass_kernel_spmd(nc, in_maps, core_ids=list(range(B)))
    # device output is (w,h)-major; unpermute on host
    out = np.stack([res.results[b]['out'].reshape(C, W, H).transpose(0, 2, 1)
                    for b in range(B)], 0)
    return out.astype(np.float32)


# revision 83
# speedup vs baseline: 1.0083x; 1.0083x over previous
"""CrossScaleSelectiveScan Trainium2 Bass kernel.

Sharding: data-parallel over batch B=8 -> one batch per NeuronCore.
Per core: bilinear resizes folded into 1x1-conv matmuls (separable
tap kernels as strided-view matmul accumulation, exact edges via
scaled-identity corrections), two 64-step GRU scans run as independent
latency-chains (vertical + horizontal), then gating + output projection
+ residual. All matmuls bf16 with fp32 PSUM accumulation; gate biases
ride the ACT bias / scalar_tensor_tensor scalar slots.

Schedule notes (cost-model driven):
- the critical path is the W scan: its step t consumes COLUMN t of x
  (all 64 rows), so x is produced twice-over: rows 0-31 as row-chunks
  (feeding early H steps while l streams in), rows 32-63 as 8-column
  PANELS so panel 0 completes soon after the last l tile lands and
  the W chain starts ~36us in (vs ~49us when x was row-only).
- inputs are cast to bf16 on HOST, so every load is cast-free and runs
  on the SP HWDGE queue (serial DMA device, order = critical path:
  P1a chunk weights, l0, rest of head weights, m0, s, bias, H-scan
  weights, l1, m1, l2, l3, m2, m3, W-scan weights, P3 weights).
  Pool never does DMA descriptor generation.
- the head interleaves P1a/P1b/x row-chunks by l-tile arrival and
  weaves in the first LEAD H steps; the u-loop pairs H step u with
  W step u-LEAD and weaves the x panels + late gxn chunks + P3 stages.
- PSUM evacuations are spread across engines: P1b copies and late-odd
  P1a copies ride ACT (Copy / Identity+bias), the rest DVE, so neither
  engine starves the copy stream that gates the panels.
- each scan keeps three separate single-buffer PSUM tiles (Pr/Pz/Pn):
  sharing a tile couples the next step's matmuls to the slack-scheduled
  sigmoid(z) read (tile-granular WAR) and adds ~350ns/step.
- out tiles are f32 so the output DMAs ride SP as well.
- deep tile-pool rotation (sc=32/sc2=24) removes WAR stalls from the
  scan chains.
"""
import numpy as np
import ml_dtypes
from contextlib import ExitStack

import concourse.bacc as bacc
import concourse.bass as bass
import concourse.mybir as mybir
import concourse.tile as tile
from concourse.bass_utils import run_bass_kernel_spmd

BF = mybir.dt.bfloat16
F32 = mybir.dt.float32
AF = mybir.ActivationFunctionType
ALU = mybir.AluOpType
NP_BF16 = ml_dtypes.bfloat16

C = 128
H = W = 64
T = 64
PX = H * W          # 4096
HL = WL = 128       # l spatial
HS = WS = 32        # s spatial
NT = 27             # weight tiles in bundle

# weight-bundle tile indices (grouped by DMA slice)
WL38, WL18, WL37, WL17 = 0, 1, 2, 3
WS34, WS14, WM = 4, 5, 6
ID18, ID38, ID356, ID156, ID34, ID14 = 7, 8, 9, 10, 11, 12
NA = 13                 # wbA tile count (head weights)
WIH_H, WHH_H = 13, 16   # +0 r, +1 z, +2 n
WIH_W, WHH_W = 19, 22
GW, PO = 25, 26

# bias columns
B_SHIFT_IN, B_GATE, B_SHIFT_OUT = 0, 1, 2
B_R_H, B_Z_H, B_HHN_H, B_IHN_H = 3, 4, 5, 6
B_R_W, B_Z_W, B_HHN_W, B_IHN_W = 7, 8, 9, 10
NB = 11


def _prep_shared(inp):
    """Build the per-core weight bundle (identical on every core)."""
    f = np.float32
    scale_i = inp['proj_in_scale'].astype(f)
    w_in = inp['proj_in_w'].astype(f) * scale_i[:, None]
    Wl, Wm, Ws = w_in[:, :C], w_in[:, C:2 * C], w_in[:, 2 * C:]
    eye = np.eye(C, dtype=f)

    tiles = [None] * NT
    for idx, k in ((WL18, 1 / 8), (WL38, 3 / 8), (WL37, 3 / 7), (WL17, 1 / 7)):
        tiles[idx] = (k * Wl).T
    for idx, k in ((ID18, 1 / 8), (ID38, 3 / 8), (ID356, 3 / 56),
                   (ID156, 1 / 56), (ID34, 3 / 4), (ID14, 1 / 4)):
        tiles[idx] = k * eye
    for idx, k in ((WS34, 3 / 4), (WS14, 1 / 4)):
        tiles[idx] = (k * Ws).T
    tiles[WM] = Wm.T
    for base, wname in ((WIH_H, 'wih_h'), (WHH_H, 'whh_h'),
                        (WIH_W, 'wih_w'), (WHH_W, 'whh_w')):
        wmat = inp[wname].astype(f)                          # [3C, C]
        for g in range(3):
            tiles[base + g] = wmat[g * C:(g + 1) * C, :].T
    tiles[GW] = inp['gate_w'].astype(f).T
    tiles[PO] = (inp['proj_out_w'].astype(f)
                 * inp['proj_out_scale'].astype(f)[:, None]).T

    wb = np.concatenate(tiles, axis=1).astype(NP_BF16)       # [128, NT*128]

    bih_h, bhh_h = inp['bih_h'].astype(f), inp['bhh_h'].astype(f)
    bih_w, bhh_w = inp['bih_w'].astype(f), inp['bhh_w'].astype(f)
    bias = np.zeros((C, NB), f)
    bias[:, B_SHIFT_IN] = inp['proj_in_shift'].astype(f)
    bias[:, B_GATE] = inp['gate_b'].astype(f)
    bias[:, B_SHIFT_OUT] = inp['proj_out_shift'].astype(f)
    bias[:, B_R_H] = bih_h[:C] + bhh_h[:C]
    bias[:, B_Z_H] = bih_h[C:2 * C] + bhh_h[C:2 * C]
    bias[:, B_HHN_H] = bhh_h[2 * C:]
    bias[:, B_IHN_H] = bih_h[2 * C:]
    bias[:, B_R_W] = bih_w[:C] + bhh_w[:C]
    bias[:, B_Z_W] = bih_w[C:2 * C] + bhh_w[C:2 * C]
    bias[:, B_HHN_W] = bhh_w[2 * C:]
    bias[:, B_IHN_W] = bih_w[2 * C:]
    return wb, bias


def build_nc(loop_n=1):
    nc = bacc.Bacc("TRN2", target_bir_lowering=False)
    l_d = nc.dram_tensor("l", [C, HL * WL], BF, kind="ExternalInput")
    m_d = nc.dram_tensor("m", [C, PX], BF, kind="ExternalInput")
    s_d = nc.dram_tensor("s", [C, HS * WS], BF, kind="ExternalInput")
    wb_d = nc.dram_tensor("wb", [C, NT * C], BF, kind="ExternalInput")
    bias_d = nc.dram_tensor("bias", [C, NB], F32, kind="ExternalInput")
    out_d = nc.dram_tensor("out", [C, PX], F32, kind="ExternalOutput")

    with tile.TileContext(nc) as tc, ExitStack() as ctx:
        big = ctx.enter_context(tc.tile_pool(name="big", bufs=1))
        l_ts = [big.tile([C, 32 * WL], BF, name=f"lt{i}", tag=f"l{i}")
                for i in range(4)]
        m_sb = big.tile([C, PX], BF, tag="m")
        s_sb = big.tile([C, HS * WS], BF, tag="s")
        wb = big.tile([C, NT * C], BF, tag="wb")
        bias = big.tile([C, NB], F32, tag="bias")
        o1 = big.tile([C, HL * W], BF, tag="o1")      # (h=128, w'=64)
        o1s = big.tile([C, HS * W], BF, tag="o1s")    # (hs=32, w'=64)
        x_sb = big.tile([C, PX], BF, tag="x")
        gxn_t = {('h', c): big.tile([C, 512], BF, name=f"gxnh{c}", tag=f"gxnh{c}")
                 for c in range(8)}
        gxn_t.update({('w', c): big.tile([C, 512], BF, name=f"gxnw{c}", tag=f"gxnw{c}")
                      for c in range(8)})
        oh = big.tile([C, PX], BF, tag="oh")          # (h, w)
        ow_ts = [big.tile([C, 512], BF, name=f"owt{i}", tag=f"owt{i}")
                 for i in range(8)]                   # (w, h) in 8 w-chunks
        h0 = big.tile([C, 64], BF, tag="h0")
        scd = big.tile([C, PX], BF, tag="scd")
        out_ts = [big.tile([C, 1024], F32, name=f"outsb{i}", tag=f"outsb{i}")
                  for i in range(4)]

        nc.vector.memset(h0[:], 0.0)
        actwarm = big.tile([C, 3], BF, tag="actwarm")
        nc.scalar.activation(actwarm[:, 0:1], h0[:, 0:1], AF.Sigmoid)
        nc.scalar.activation(actwarm[:, 1:2], h0[:, 1:2], AF.Tanh)
        nc.scalar.activation(actwarm[:, 2:3], h0[:, 2:3], AF.Relu)

        def wt(i):
            return wb[:, i * C:(i + 1) * C]

        def bcol(i):
            return bias[:, i:i + 1]

        sv = s_sb[:].rearrange("p (h w) -> p h w", h=HS)
        o1v = o1[:].rearrange("p (h w) -> p h w", h=HL)
        o1sv = o1s[:].rearrange("p (h w) -> p h w", h=HS)
        xv = x_sb[:].rearrange("p (h w) -> p h w", h=H)
        mv = m_sb[:].rearrange("p (h w) -> p h w", h=H)
        mm = nc.tensor.matmul
        dma = nc.sync.dma_start

        for _it in range(loop_n):
            # ---- loads: cast-free (host pre-casts to bf16) on the SP HWDGE
            # queue. The DMA device is serial in the model, so ORDER = the
            # critical path.
            dma(wb[:, 0:2 * C], wb_d[:, 0:2 * C])     # P1a chunk weights
            dma(l_ts[0][:], l_d[:, 0:4096])
            dma(wb[:, 2 * C:NA * C], wb_d[:, 2 * C:NA * C])
            dma(m_sb[:, 0:1024], m_d[:, 0:1024])
            dma(s_sb[:], s_d[:])
            dma(bias[:], bias_d[:])
            dma(wb[:, NA * C:(NA + 6) * C], wb_d[:, NA * C:(NA + 6) * C])
            dma(l_ts[1][:], l_d[:, 4096:8192])
            dma(m_sb[:, 1024:2048], m_d[:, 1024:2048])
            dma(l_ts[2][:], l_d[:, 8192:12288])
            dma(l_ts[3][:], l_d[:, 12288:16384])
            dma(m_sb[:, 2048:3072], m_d[:, 2048:3072])
            dma(m_sb[:, 3072:4096], m_d[:, 3072:4096])
            dma(wb[:, (NA + 6) * C:(NA + 12) * C],
                wb_d[:, (NA + 6) * C:(NA + 12) * C])
            dma(wb[:, (NA + 12) * C:], wb_d[:, (NA + 12) * C:])
            lvs = [t[:].rearrange("p (h w) -> p h w", h=32) for t in l_ts]

            LEAD = 7

            with tc.tile_pool(name="psn", bufs=1, space="PSUM") as psn, \
                 tc.tile_pool(name="sc", bufs=32) as sc, \
                 tc.tile_pool(name="sc2", bufs=24) as sc2, \
                 tc.tile_pool(name="p3", bufs=4) as p3:
                psH_cm = tc.tile_pool(name="psH", bufs=1, space="PSUM")
                psH = psH_cm.__enter__()
                P = {}   # late-bound pools (psx/ps1/pse in head, psW/p3d after)

                def p1a_chunk(k):
                    lt = lvs[k // 4]
                    r0 = 8 * (k % 4)
                    p = P['ps1'].tile([C, 512], F32, tag="o1p", name=f"o1p{k}")
                    pvv = p[:].rearrange("p (h w) -> p h w", h=8)
                    rows = lt[:, r0:r0 + 8, :]
                    mm(pvv[:, :, :], wt(WL38), rows[:, :, 0:128:2], start=True, stop=False, skip_group_check=True)
                    mm(pvv[:, :, :], wt(WL38), rows[:, :, 1:128:2], start=False, stop=False, skip_group_check=True)
                    mm(pvv[:, :, 1:64], wt(WL18), rows[:, :, 1:126:2], start=False, stop=False, skip_group_check=True)
                    mm(pvv[:, :, 0:63], wt(WL18), rows[:, :, 2:127:2], start=False, stop=True, skip_group_check=True)
                    # copy only cols 1..62: cols 0/63 belong to the exact
                    # edge pass, whose copies are order-independent this way.
                    # Late chunks alternate DVE/ACT so the copy stream (which
                    # gates the x panels and the W scan) is not DVE-bound.
                    if k >= 8 and k % 2 == 1:
                        nc.scalar.activation(o1v[:, 8 * k:8 * k + 8, 1:63],
                                             pvv[:, :, 1:63], AF.Copy)
                    else:
                        nc.vector.tensor_copy(o1v[:, 8 * k:8 * k + 8, 1:63],
                                              pvv[:, :, 1:63])

                def p1a_edge(lti):
                    # exact edge columns w'=0 / w'=63 for l-tile lti's rows
                    ep_t = P['pse'].tile([C, 512], F32, tag="ep", name=f"ep{lti}")
                    epv = ep_t[:, 0:256].rearrange("p (e h) -> p e h", e=2)
                    for i, (wcol, widx) in enumerate(((0, WL37), (1, WL37), (2, WL17))):
                        mm(epv[:, 0, 32 * lti:32 * lti + 32], wt(widx),
                           lvs[lti][:, :, wcol], start=(i == 0),
                           stop=(i == 2), skip_group_check=True)
                    for i, (wcol, widx) in enumerate(((125, WL17), (126, WL37), (127, WL37))):
                        mm(epv[:, 1, 32 * lti:32 * lti + 32], wt(widx),
                           lvs[lti][:, :, wcol], start=(i == 0),
                           stop=(i == 2), skip_group_check=True)
                    nc.vector.tensor_copy(o1v[:, 32 * lti:32 * lti + 32, 0],
                                          epv[:, 0, 32 * lti:32 * lti + 32])
                    nc.vector.tensor_copy(o1v[:, 32 * lti:32 * lti + 32, 63],
                                          epv[:, 1, 32 * lti:32 * lti + 32])

                def p1b_chunk(k):
                    # s W-upsample fused with conv -> o1s rows 8k..8k+7
                    r0 = 8 * k
                    p = P['pse'].tile([C, 512], F32, tag="ep", name=f"o1sp{k}")
                    pvv = p[:].rearrange("p (h w) -> p h w", h=8)
                    srows = sv[:, r0:r0 + 8, :]
                    mm(pvv[:, :, 0:64:2], wt(WS34), srows[:, :, 0:32], start=True, stop=False, skip_group_check=True)
                    mm(pvv[:, :, 1:64:2], wt(WS34), srows[:, :, 0:32], start=False, stop=False, skip_group_check=True)
                    mm(pvv[:, :, 2:64:2], wt(WS14), srows[:, :, 0:31], start=False, stop=False, skip_group_check=True)
                    mm(pvv[:, :, 1:63:2], wt(WS14), srows[:, :, 1:32], start=False, stop=False, skip_group_check=True)
                    mm(pvv[:, :, 0], wt(WS14), srows[:, :, 0], start=False, stop=False, skip_group_check=True)
                    mm(pvv[:, :, 63], wt(WS14), srows[:, :, 31], start=False, stop=True, skip_group_check=True)
                    # PSUM evacuation on ACT: keeps the head DVE free for
                    # the o1p copies that gate the x panels
                    nc.scalar.activation(o1s[:, k * 512:(k + 1) * 512], p[:],
                                         AF.Copy)

                def x_chunk(c):
                    # x rows 8c..8c+7, all 64 cols (c = 0..3 only); PSUM
                    # rides the psn ring (alternates with its gxn chunk)
                    hp0 = 8 * c
                    p = psn.tile([C, 512], F32, tag="gxnp", name=f"xp{c}")
                    pvv = p[:].rearrange("p (h w) -> p h w", h=8)
                    mm(p[:], wt(WM), m_sb[:, c * 512:(c + 1) * 512], start=True, stop=False, skip_group_check=True)
                    mm(pvv[:, :, :], wt(ID38), o1v[:, 2 * hp0:2 * hp0 + 16:2, :], start=False, stop=False, skip_group_check=True)
                    mm(pvv[:, :, :], wt(ID38), o1v[:, 2 * hp0 + 1:2 * hp0 + 16:2, :], start=False, stop=False, skip_group_check=True)
                    if c == 0:
                        mm(pvv[:, 1:8, :], wt(ID18), o1v[:, 1:15:2, :], start=False, stop=False, skip_group_check=True)
                    else:
                        mm(pvv[:, :, :], wt(ID18), o1v[:, 2 * hp0 - 1:2 * hp0 + 15:2, :], start=False, stop=False, skip_group_check=True)
                    mm(pvv[:, :, :], wt(ID18), o1v[:, 2 * hp0 + 2:2 * hp0 + 18:2, :], start=False, stop=False, skip_group_check=True)
                    if c == 0:
                        mm(pvv[:, 0, :], wt(ID356), o1v[:, 0, :], start=False, stop=False, skip_group_check=True)
                        mm(pvv[:, 0, :], wt(ID356), o1v[:, 1, :], start=False, stop=False, skip_group_check=True)
                        mm(pvv[:, 0, :], wt(ID156), o1v[:, 2, :], start=False, stop=False, skip_group_check=True)
                    p0 = 4 * c
                    mm(pvv[:, 0:8:2, :], wt(ID34), o1sv[:, p0:p0 + 4, :], start=False, stop=False, skip_group_check=True)
                    mm(pvv[:, 1:8:2, :], wt(ID34), o1sv[:, p0:p0 + 4, :], start=False, stop=False, skip_group_check=True)
                    if c == 0:
                        mm(pvv[:, 2:8:2, :], wt(ID14), o1sv[:, 0:3, :], start=False, stop=False, skip_group_check=True)
                        mm(pvv[:, 0, :], wt(ID14), o1sv[:, 0, :], start=False, stop=False, skip_group_check=True)
                    else:
                        mm(pvv[:, 0:8:2, :], wt(ID14), o1sv[:, p0 - 1:p0 + 3, :], start=False, stop=False, skip_group_check=True)
                    mm(pvv[:, 1:8:2, :], wt(ID14), o1sv[:, p0 + 1:p0 + 5, :], start=False, stop=(c != 0), skip_group_check=True)
                    nc.scalar.activation(x_sb[:, c * 512:(c + 1) * 512], p[:],
                                         AF.Relu, bias=bcol(B_SHIFT_IN))
                    # gxn for the H scan streams right behind each x chunk;
                    # the bias-add rides ACT (Identity+bias) in the head
                    pg = psn.tile([C, 512], F32, tag="gxnp", name=f"gxnp{c}")
                    mm(pg[:], wt(WIH_H + 2), x_sb[:, c * 512:(c + 1) * 512],
                       start=True, stop=True, skip_group_check=True)
                    nc.scalar.activation(gxn_t[('h', c)][:], pg[:],
                                         AF.Identity, bias=bcol(B_IHN_H))

                def gxn_h_chunk(c):
                    # late H gxn chunks (rows from the x panels), c = 4..7;
                    # DVE halves (ACT carries both chains in the u-loop)
                    pg = psn.tile([C, 512], F32, tag="gxnp", name=f"gxnp{c}")
                    mm(pg[:], wt(WIH_H + 2), x_sb[:, c * 512:(c + 1) * 512],
                       start=True, stop=True, skip_group_check=True)
                    for hh in range(2):
                        nc.vector.tensor_scalar(
                            gxn_t[('h', c)][:, hh * 256:(hh + 1) * 256],
                            pg[:, hh * 256:(hh + 1) * 256],
                            bcol(B_IHN_H), None, ALU.add)

                def x_panel(p):
                    # x rows 32..63 for cols 8p..8p+8 (feeds W steps early);
                    # PSUM rides the psn ring (alternates with gxn chunks)
                    j0 = 8 * p
                    pp = psn.tile([C, 512], F32, tag="gxnp", name=f"xpp{p}")
                    pv = pp[:, 0:256].rearrange("p (h w) -> p h w", h=32)
                    mm(pv[:, :, :], wt(WM), mv[:, 32:64, j0:j0 + 8], start=True, stop=False, skip_group_check=True)
                    mm(pv[:, :, :], wt(ID38), o1v[:, 64:128:2, j0:j0 + 8], start=False, stop=False, skip_group_check=True)
                    mm(pv[:, :, :], wt(ID38), o1v[:, 65:128:2, j0:j0 + 8], start=False, stop=False, skip_group_check=True)
                    mm(pv[:, :, :], wt(ID18), o1v[:, 63:127:2, j0:j0 + 8], start=False, stop=False, skip_group_check=True)
                    mm(pv[:, 0:31, :], wt(ID18), o1v[:, 66:128:2, j0:j0 + 8], start=False, stop=False, skip_group_check=True)
                    mm(pv[:, 31, :], wt(ID156), o1v[:, 125, j0:j0 + 8], start=False, stop=False, skip_group_check=True)
                    mm(pv[:, 31, :], wt(ID356), o1v[:, 126, j0:j0 + 8], start=False, stop=False, skip_group_check=True)
                    mm(pv[:, 31, :], wt(ID356), o1v[:, 127, j0:j0 + 8], start=False, stop=False, skip_group_check=True)
                    mm(pv[:, 0:32:2, :], wt(ID34), o1sv[:, 16:32, j0:j0 + 8], start=False, stop=False, skip_group_check=True)
                    mm(pv[:, 1:32:2, :], wt(ID34), o1sv[:, 16:32, j0:j0 + 8], start=False, stop=False, skip_group_check=True)
                    mm(pv[:, 0:32:2, :], wt(ID14), o1sv[:, 15:31, j0:j0 + 8], start=False, stop=False, skip_group_check=True)
                    mm(pv[:, 1:31:2, :], wt(ID14), o1sv[:, 17:32, j0:j0 + 8], start=False, stop=False, skip_group_check=True)
                    mm(pv[:, 31, :], wt(ID14), o1sv[:, 31, j0:j0 + 8], start=False, stop=True, skip_group_check=True)
                    nc.scalar.activation(xv[:, 32:64, j0:j0 + 8], pv[:, :, :],
                                         AF.Relu, bias=bcol(B_SHIFT_IN))

                def gxn_w_chunk(c):
                    pg = psn.tile([C, 512], F32, tag="gxnp", name=f"gxnw{c}")
                    rhs = xv[:, :, 8 * c:8 * c + 8].transpose([0, 2, 1])
                    mm(pg[:].rearrange("p (w h) -> p w h", w=8), wt(WIH_W + 2), rhs,
                       start=True, stop=True, skip_group_check=True)
                    # bias-add in [C,256] halves (GPSIMD has no PSUM port, so
                    # these stay on DVE; halves bound the chain-delay spill)
                    for hh in range(2):
                        nc.vector.tensor_scalar(
                            gxn_t[('w', c)][:, hh * 256:(hh + 1) * 256],
                            pg[:, hh * 256:(hh + 1) * 256],
                            bcol(B_IHN_W), None, ALU.add)

                scans = {
                    'h': ('h', WIH_H, WHH_H, B_R_H, B_Z_H, B_HHN_H),
                    'w': ('w', WIH_W, WHH_W, B_R_W, B_Z_W, B_HHN_W),
                }
                spool = {'h': psH}

                def hbuf(sname, t):
                    if sname == 'h':
                        return oh[:, t * 64:(t + 1) * 64]
                    return ow_ts[t // 8][:, (t % 8) * 64:(t % 8) * 64 + 64]
                Pcur = {}
                Pcur2 = {}
                prev = {'h': None, 'w': None}

                def scan_mms(sname, t):
                    buf, wih, whh = scans[sname][:3]
                    pool = spool[sname]
                    Pr_t = pool.tile([C, 64], F32, tag=f"Pr{sname}", name=f"Pr{sname}{t}")
                    Pz_t = pool.tile([C, 64], F32, tag=f"Pz{sname}", name=f"Pz{sname}{t}")
                    Pn_t = pool.tile([C, 64], F32, tag=f"Pn{sname}", name=f"Pn{sname}{t}")
                    Pr, Pz, Pn = Pr_t[:], Pz_t[:], Pn_t[:]
                    xt = (x_sb[:, t * 64:(t + 1) * 64]
                          if sname == 'h' else xv[:, :, t])
                    mm(Pr, wt(wih + 0), xt, start=True, stop=False, skip_group_check=True)
                    mm(Pz, wt(wih + 1), xt, start=True, stop=False, skip_group_check=True)
                    if prev[sname] is None:
                        mm(Pr, wt(whh + 0), h0[:], start=False, stop=True, skip_group_check=True)
                        mm(Pn, wt(whh + 2), h0[:], start=True, stop=True, skip_group_check=True)
                        mm(Pz, wt(whh + 1), h0[:], start=False, stop=True, skip_group_check=True)
                    else:
                        # h' = t1 + t2, and whh@h' = whh@t2 + whh@t1: the t2
                        # half issues early (t2 is ready before tanh), only
                        # the t1 half waits on the tanh chain.
                        t1p, t2p = prev[sname]
                        mm(Pr, wt(whh + 0), t2p[:], start=False, stop=False, skip_group_check=True)
                        mm(Pn, wt(whh + 2), t2p[:], start=True, stop=False, skip_group_check=True)
                        mm(Pz, wt(whh + 1), t2p[:], start=False, stop=False, skip_group_check=True)
                        mm(Pr, wt(whh + 0), t1p[:], start=False, stop=True, skip_group_check=True)
                        mm(Pn, wt(whh + 2), t1p[:], start=False, stop=True, skip_group_check=True)
                        mm(Pz, wt(whh + 1), t1p[:], start=False, stop=True, skip_group_check=True)
                    Pcur[sname] = (Pr, Pz, Pn)

                def scan_gates(sname, t):
                    buf, wih, whh, br, bz, bhhn = scans[sname]
                    Pr, Pz, Pn = Pcur[sname]
                    hp = h0[:] if t == 0 else hbuf(sname, t - 1)
                    r = sc.tile([C, 64], BF, tag=f"r{sname}", name=f"r{sname}{t}")
                    nc.scalar.activation(r[:], Pr, AF.Sigmoid, bias=bcol(br))
                    q = sc.tile([C, 64], BF, tag=f"q{sname}", name=f"q{sname}{t}")
                    nc.vector.scalar_tensor_tensor(q[:], Pn,
                                                   bcol(bhhn), r[:],
                                                   ALU.add, ALU.mult)
                    nin = sc.tile([C, 64], BF, tag=f"nin{sname}", name=f"nin{sname}{t}")
                    nc.vector.tensor_add(nin[:], q[:],
                                         gxn_t[(sname, t // 8)][:, (t % 8) * 64:(t % 8) * 64 + 64])
                    # sigma_z issues BEFORE tanh: it executes in the ACT
                    # gap while the STT/nin legs run, so zc (which gates the
                    # on-chain t1 multiply) is ready by the time tanh lands.
                    z = sc2.tile([C, 64], BF, tag=f"z{sname}", name=f"z{sname}{t}")
                    nc.scalar.activation(z[:], Pz, AF.Sigmoid, bias=bcol(bz))
                    zc = sc2.tile([C, 64], BF, tag=f"zc{sname}", name=f"zc{sname}{t}")
                    nc.gpsimd.tensor_scalar(zc[:], z[:], -1.0, 1.0,
                                            ALU.mult, ALU.add)
                    t2 = sc2.tile([C, 64], BF, tag=f"t2{sname}", name=f"t2{sname}{t}")
                    nc.gpsimd.tensor_mul(t2[:], z[:], hp)
                    n = sc.tile([C, 64], BF, tag=f"n{sname}", name=f"n{sname}{t}")
                    nc.scalar.activation(n[:], nin[:], AF.Tanh)
                    t1 = sc.tile([C, 64], BF, tag=f"t1{sname}", name=f"t1{sname}{t}")
                    nc.vector.tensor_mul(t1[:], n[:], zc[:])
                    nc.vector.tensor_add(hbuf(sname, t), t1[:], t2[:])
                    prev[sname] = (t1, t2)

                # ---- P3 stage helpers (issued in the W-tail + drain)
                ohv = oh[:].rearrange("p (h w) -> p h w", h=H)
                scdv = scd[:].rearrange("p (w h) -> p w h", w=W)

                def p3_stage0(c):
                    # scd = ow + oh^T in [C,256] halves, split DVE/Pool
                    for hh in range(2):
                        h2 = slice(c * 512 + hh * 256, c * 512 + (hh + 1) * 256)
                        oh_view = ohv[:, :, 8 * c + 4 * hh:8 * c + 4 * hh + 4] \
                            .transpose([0, 2, 1])
                        eng = nc.vector if hh == 0 else nc.gpsimd
                        eng.tensor_add(scd[:, h2],
                                       ow_ts[c][:, hh * 256:(hh + 1) * 256],
                                       oh_view)

                def p3_stage1(c):
                    # gp = GW@ow + GW@oh^T accumulated in PSUM: the gate path
                    # does not wait for the Pool scd add (scd only feeds the
                    # gated multiply in stage2)
                    gp = P['p3gp'].tile([C, 512], F32, tag="p3gp", name=f"gp{c}")
                    g = p3.tile([C, 512], BF, tag="g", name=f"g{c}")
                    for hh in range(2):
                        h2 = slice(hh * 256, (hh + 1) * 256)
                        oh_view = ohv[:, :, 8 * c + 4 * hh:8 * c + 4 * hh + 4] \
                            .transpose([0, 2, 1])
                        mm(gp[:, h2], wt(GW), ow_ts[c][:, hh * 256:(hh + 1) * 256],
                           start=True, stop=False, skip_group_check=True)
                        mm(gp[:, h2].rearrange("p (w h) -> p w h", w=4), wt(GW),
                           oh_view, start=False, stop=True, skip_group_check=True)
                        nc.scalar.activation(g[:, h2], gp[:, h2], AF.Sigmoid,
                                             bias=bcol(B_GATE))
                    return g

                def p3_stage2(c, g):
                    sl = slice(c * 512, (c + 1) * 512)
                    gated = p3.tile([C, 512], BF, tag="gated", name=f"gated{c}")
                    op = psn.tile([C, 512], F32, tag="gxnp", name=f"op{c}")
                    for hh in range(2):
                        h2 = slice(hh * 256, (hh + 1) * 256)
                        nc.vector.tensor_mul(gated[:, h2], scd[:, sl][:, h2], g[:, h2])
                        mm(op[:, h2], wt(PO), gated[:, h2],
                           start=True, stop=True, skip_group_check=True)
                    return op

                def p3_stage3(c, op):
                    y = p3.tile([C, 512], BF, tag="y", name=f"y{c}")
                    nc.vector.tensor_scalar(y[:, 0:256], op[:, 0:256],
                                            bcol(B_SHIFT_OUT), 0.0,
                                            ALU.add, ALU.max)
                    nc.scalar.activation(y[:, 256:512], op[:, 256:512],
                                         AF.Relu, bias=bcol(B_SHIFT_OUT))
                    return y

                def p3_cols(w0, w1):
                    # all P3 stages for w-cols [w0, w1) in one serial strip
                    # (used for the last chunk so only col 63 drains at the end)
                    nw = w1 - w0
                    sl = slice(w0 * 64, w1 * 64)
                    ow_v = ow_ts[w0 // 8][:, (w0 % 8) * 64:(w0 % 8) * 64 + nw * 64]
                    oh_view = ohv[:, :, w0:w1].transpose([0, 2, 1])
                    eng = nc.vector if (w0 % 2 == 0) else nc.gpsimd
                    eng.tensor_add(scd[:, sl], ow_v, oh_view)
                    gp = P['p3gp'].tile([C, 512], F32, tag="p3gp", name=f"gpc{w0}")
                    g = p3.tile([C, 512], BF, tag="g", name=f"gc{w0}")
                    mm(gp[:, 0:nw * 64], wt(GW), ow_v,
                       start=True, stop=False, skip_group_check=True)
                    mm(gp[:, 0:nw * 64].rearrange("p (w h) -> p w h", w=nw), wt(GW),
                       oh_view, start=False, stop=True, skip_group_check=True)
                    nc.scalar.activation(g[:, 0:nw * 64], gp[:, 0:nw * 64],
                                         AF.Sigmoid, bias=bcol(B_GATE))
                    gated = p3.tile([C, 512], BF, tag="gated", name=f"gatedc{w0}")
                    nc.vector.tensor_mul(gated[:, 0:nw * 64], scd[:, sl],
                                         g[:, 0:nw * 64])
                    op = psn.tile([C, 512], F32, tag="gxnp", name=f"opc{w0}")
                    mm(op[:, 0:nw * 64], wt(PO), gated[:, 0:nw * 64],
                       start=True, stop=True, skip_group_check=True)
                    y = p3.tile([C, 512], BF, tag="y", name=f"yc{w0}")
                    nc.vector.tensor_scalar(y[:, 0:nw * 64], op[:, 0:nw * 64],
                                            bcol(B_SHIFT_OUT), 0.0,
                                            ALU.add, ALU.max)
                    x_view = xv[:, :, w0:w1].transpose([0, 2, 1])
                    ot = out_ts[w0 // 16][:, (w0 % 16) * 64:(w0 % 16) * 64 + nw * 64]
                    nc.vector.tensor_add(ot, y[:, 0:nw * 64], x_view)
                    dma(out_d[:, sl], ot)

                def p3_stage4(c, y):
                    for hh in range(2):
                        h2 = slice(hh * 256, (hh + 1) * 256)
                        x_view = xv[:, :, 8 * c + 4 * hh:8 * c + 4 * hh + 4] \
                            .transpose([0, 2, 1])
                        nc.vector.tensor_add(
                            out_ts[c // 2][:, (c % 2) * 512 + hh * 256:
                                           (c % 2) * 512 + (hh + 1) * 256],
                            y[:, h2], x_view)
                    dma(out_d[:, c * 512:(c + 1) * 512],
                        out_ts[c // 2][:, (c % 2) * 512:(c % 2) * 512 + 512])

                # ===== HEAD: P1a/P1b/x row-chunks interleaved by l-tile
                # arrival, first LEAD H steps woven in.
                with tc.tile_pool(name="ps1", bufs=3, space="PSUM") as ps1, \
                     tc.tile_pool(name="pse", bufs=1, space="PSUM") as pse:
                    P['ps1'], P['pse'] = ps1, pse
                    # PE p-state warmup: the cost model needs ~3us of
                    # continuous PE busy to reach full clock; spin on h0
                    # until the l0-dependent work arrives.
                    warm = ps1.tile([C, 512], F32, tag="o1p", name="warm")
                    for _w in range(140):
                        mm(warm[:][0:64, 0:64], h0[:, 0:64], h0[:],
                           start=True, stop=True, skip_group_check=True)

                    # l0-gated work + first H steps
                    for k in range(4):
                        p1a_chunk(k)
                    p1a_edge(0)
                    p1b_chunk(0)
                    p1b_chunk(1)
                    x_chunk(0)
                    for t in (0, 1):
                        with tc.high_priority(offset=100000):
                            scan_mms('h', t)
                            scan_gates('h', t)
                    # l1-gated
                    for k in range(4, 8):
                        p1a_chunk(k)
                    p1a_edge(1)
                    x_chunk(1)
                    for t in (2, 3):
                        with tc.high_priority(offset=100000):
                            scan_mms('h', t)
                            scan_gates('h', t)
                    p1b_chunk(2)
                    p1b_chunk(3)
                    x_chunk(2)
                    # l2-gated
                    for k in range(8, 12):
                        p1a_chunk(k)
                    p1a_edge(2)
                    x_chunk(3)
                    for t in (4, 5):
                        with tc.high_priority(offset=100000):
                            scan_mms('h', t)
                            scan_gates('h', t)
                    # l3-gated
                    for k in range(12, 16):
                        p1a_chunk(k)
                    p1a_edge(3)
                    x_panel(0)
                    # the u-loop starts at u=LEAD: every H step below LEAD
                    # must be woven here or it would never be issued
                    for t in range(6, LEAD):
                        with tc.high_priority(offset=100000):
                            scan_mms('h', t)
                            scan_gates('h', t)
                    gxn_w_chunk(0)

                # ===== u-loop: paired H+W steps; x panels, late gxn chunks,
                # early scd halves and P3 stages woven in.
                with tc.tile_pool(name="psW", bufs=1, space="PSUM") as psW, \
                     tc.tile_pool(name="p3gp", bufs=1, space="PSUM") as p3gp:
                    spool['w'] = psW
                    P['p3gp'] = p3gp

                    # weave schedule: panel p (p>=1) at u, late gxn_h at u
                    u0 = max(LEAD, 5)
                    panel_at = {u0 + 2 * p: p for p in range(1, 8)}
                    gxnh_at = {u0 + 16 + 2 * i: 4 + i for i in range(4)}

                    # P3: chunk c needs the FULL H-scan (u >= T) plus ow
                    # chunk c (u >= 8c+7+LEAD). Whatever does not fit in the
                    # u-loop window runs in the stage-major drain.
                    p3_sched = {}          # u -> list of (stage_idx, chunk)
                    p3_done = set()
                    for c3 in range(7):
                        for s in range(5):
                            uu = max(T, 8 * c3 + 7 + LEAD) + 2 * c3 + s
                            if uu < T + LEAD:
                                p3_sched.setdefault(uu, []).append((s, c3))
                                p3_done.add((s, c3))

                    p3g = {}
                    for u in range(LEAD, T + LEAD):
                        tH, tW = u, u - LEAD
                        with tc.high_priority(offset=100000):
                            if tH < T:
                                scan_mms('h', tH)
                            scan_mms('w', tW)
                            if tH < T:
                                scan_gates('h', tH)
                            scan_gates('w', tW)
                        if u in panel_at:
                            x_panel(panel_at[u])
                            gxn_w_chunk(panel_at[u])
                        if u in gxnh_at:
                            gxn_h_chunk(gxnh_at[u])
                        for (s, c3) in p3_sched.get(u, []):
                            if s == 0:
                                p3_stage0(c3)
                            elif s == 1:
                                p3g[c3] = p3_stage1(c3)
                            elif s == 2:
                                p3g[c3] = p3_stage2(c3, p3g[c3])
                            elif s == 3:
                                p3g[c3] = p3_stage3(c3, p3g[c3])
                            else:
                                p3_stage4(c3, p3g.pop(c3))
                    # stage-major drain of everything the u-loop didn't fit
                    for s in range(5):
                        for c3 in range(7):
                            if (s, c3) in p3_done:
                                continue
                            if s == 0:
                                p3_stage0(c3)
                            elif s == 1:
                                p3g[c3] = p3_stage1(c3)
                            elif s == 2:
                                p3g[c3] = p3_stage2(c3, p3g[c3])
                            elif s == 3:
                                p3g[c3] = p3_stage3(c3, p3g[c3])
                            else:
                                p3_stage4(c3, p3g.pop(c3))
                    p3_cols(56, 60)
                    p3_cols(60, 63)
                    p3_cols(63, 64)
                psH_cm.__exit__(None, None, None)

    nc.finalize()
    return nc


_NC_CACHE = {}


def kernel(**inputs):
    inputs = {k: np.asarray(v) for k, v in inputs.items()}
    B = inputs['l'].shape[0]
    wb, bias = _prep_shared(inputs)
    if 'nc' not in _NC_CACHE:
        _NC_CACHE['nc'] = build_nc()
    nc = _NC_CACHE['nc']
    in_maps = []
    for b in range(B):
        in_maps.append({
            'l': inputs['l'][b].reshape(C, -1).astype(NP_BF16),
            'm': inputs['m'][b].reshape(C, -1).astype(NP_BF16),
            's': inputs['s'][b].reshape(C, -1).astype(NP_BF16),
            'wb': wb, 'bias': bias,
        })
    res = run_bass_kernel_spmd(nc, in_maps, core_ids=list(range(B)))
    # device output is (w,h)-major; unpermute on host
    out = np.stack([res.results[b]['out'].reshape(C, W, H).transpose(0, 2, 1)
                    for b in range(B)], 0)
    return out.astype(np.float32)
